# revision 3
# baseline (speedup 1.0000x reference)
"""MoE (16 experts, top-2) + shared SwiGLU expert — Trainium2 Bass kernel.

Strategy (8 NeuronCores, SPMD):
  - Router runs on host (tiny: 2048x1024x16). Tokens are grouped by expert.
  - Expert-parallel: core c owns experts {2c, 2c+1}; host gathers the tokens
    routed to each expert (padded to capacity C) and ships them transposed
    (features-on-partitions) so no on-device transposes are needed.
  - Shared expert is FF-sharded: core c computes a 352-wide slice of the
    2816-wide SwiGLU FF; host sums the 8 partial down-projections.
  - All matmul inputs are cast to bf16 on host (fp32 accumulation in PSUM).
  - Host combine: out = sum(partials).T * sigmoid(x@sgw.T) + scatter(expert).
"""

import os

import numpy as np
import ml_dtypes

import concourse.bass as bass
import concourse.mybir as mybir
import concourse.tile as tile
from concourse.bass_utils import run_bass_kernel_spmd

HIDDEN = 1024
MOE_FF = 512
SHARED_FF = 2816
NUM_EXPERTS = 16
TOP_K = 2
N_CORES = 8
NTOK = 2048
FF_SH = SHARED_FF // N_CORES  # 352
P = 128
KD = HIDDEN // P  # 8 contraction chunks over hidden
FF_CH = [(0, 128), (128, 128), (256, 96)]  # shared-FF shard chunking (352)
TT = 512  # token tile (PSUM free-dim limit)
NT = NTOK // TT

BF16 = ml_dtypes.bfloat16

_prog_cache: dict = {}


def _split_excess_waits(nc: bass.Bass) -> None:
    """This container's walrus accepts at most 1 sync-wait per instruction
    (2 on EventSemaphore), but Tile's tail barrier can emit more; split the
    excess onto preceding EventSemaphore instructions on the same engine."""
    for fn in nc.m.functions:
        for blk in fn.blocks:
            out = []
            for ins in blk.instructions:
                si = ins.sync_info
                cap = 2 if isinstance(ins, mybir.InstEventSemaphore) else 1
                if si is not None and len(si.on_wait) > cap:
                    waits = list(si.on_wait)
                    excess, keep = waits[:-cap], waits[-cap:]
                    for i in range(0, len(excess), 2):
                        ev = mybir.InstEventSemaphore(
                            name=nc.get_next_instruction_name(), ins=[], outs=[])
                        ev.engine = ins.engine
                        ev.sync_info = mybir.SyncInfo(
                            on_wait=excess[i:i + 2], on_update=[])
                        nc.register_instruction(ev)
                        out.append(ev)
                    si.on_wait = keep
                out.append(ins)
            blk.instructions[:] = out


def _build_program(C: int) -> bass.Bass:
    """Per-core program. C = per-expert token capacity (multiple of 128)."""
    nc = bass.Bass()
    dt = mybir.dt
    f = mybir.ActivationFunctionType

    xt_d = nc.dram_tensor("xt", [HIDDEN, NTOK], dt.bfloat16, kind="ExternalInput")
    xe_d = nc.dram_tensor("xe", [HIDDEN, 2 * C], dt.bfloat16, kind="ExternalInput")
    wgu_d = nc.dram_tensor("wgu", [2, HIDDEN, 2 * MOE_FF], dt.bfloat16, kind="ExternalInput")
    wd_d = nc.dram_tensor("wd", [2, MOE_FF, HIDDEN], dt.bfloat16, kind="ExternalInput")
    wg_d = nc.dram_tensor("wg_t", [HIDDEN, FF_SH], dt.bfloat16, kind="ExternalInput")
    wu_d = nc.dram_tensor("wu_t", [HIDDEN, FF_SH], dt.bfloat16, kind="ExternalInput")
    wdt_d = nc.dram_tensor("wdt", [FF_SH, HIDDEN], dt.bfloat16, kind="ExternalInput")
    ymoe_d = nc.dram_tensor("y_moe", [HIDDEN, 2 * C], dt.float32, kind="ExternalOutput")
    ysh_d = nc.dram_tensor("y_sh", [HIDDEN, NTOK], dt.float32, kind="ExternalOutput")

    with tile.TileContext(nc) as tc:
        with (
            tc.tile_pool(name="res", bufs=1) as res,
            tc.tile_pool(name="wmoe", bufs=2) as wmoe,
            tc.tile_pool(name="acts", bufs=2) as acts,
            tc.tile_pool(name="outs", bufs=4) as outs,
            tc.tile_pool(name="psum", bufs=2, space="PSUM") as psum,
        ):
            # ---- resident loads ----
            xe = []
            for k in range(KD):
                t = res.tile([P, 2 * C], dt.bfloat16, tag=f"xe{k}", name=f"xe{k}")
                nc.sync.dma_start(t[:], xe_d[k * P:(k + 1) * P, :])
                xe.append(t)
            xt = []
            for k in range(KD):
                t = res.tile([P, NTOK], dt.bfloat16, tag=f"xt{k}", name=f"xt{k}")
                nc.sync.dma_start(t[:], xt_d[k * P:(k + 1) * P, :])
                xt.append(t)
            wg = []
            wu = []
            for k in range(KD):
                t = res.tile([P, FF_SH], dt.bfloat16, tag=f"wg{k}", name=f"wg{k}")
                nc.sync.dma_start(t[:], wg_d[k * P:(k + 1) * P, :])
                wg.append(t)
                t = res.tile([P, FF_SH], dt.bfloat16, tag=f"wu{k}", name=f"wu{k}")
                nc.sync.dma_start(t[:], wu_d[k * P:(k + 1) * P, :])
                wu.append(t)
            wdt = []
            for j, (off, sz) in enumerate(FF_CH):
                t = res.tile([P, HIDDEN], dt.bfloat16, tag=f"wdt{j}", name=f"wdt{j}")
                nc.sync.dma_start(t[:sz, :], wdt_d[off:off + sz, :])
                wdt.append(t)

            # ---- MoE experts (2 per core) ----
            for e in range(2):
                wgu = []
                for k in range(KD):
                    t = wmoe.tile([P, 2 * MOE_FF], dt.bfloat16, tag=f"wgu{k}", name=f"wgu{e}_{k}")
                    nc.sync.dma_start(t[:], wgu_d[e, k * P:(k + 1) * P, :])
                    wgu.append(t)
                wdl = []
                for j in range(MOE_FF // P):
                    t = wmoe.tile([P, HIDDEN], dt.bfloat16, tag=f"wdl{j}", name=f"wd{e}_{j}")
                    nc.sync.dma_start(t[:], wd_d[e, j * P:(j + 1) * P, :])
                    wdl.append(t)

                tok = xe  # rhs slices [:, e*C:(e+1)*C]
                hs = []
                for j in range(4):  # gate/up feature pairs (j, j+4) of 2*MOE_FF
                    pg = psum.tile([P, TT], dt.float32, tag="pg", name=f"pg{e}_{j}")
                    pu = psum.tile([P, TT], dt.float32, tag="pu", name=f"pu{e}_{j}")
                    for k in range(KD):
                        nc.tensor.matmul(
                            pg[:, :C], wgu[k][:, j * P:(j + 1) * P],
                            tok[k][:, e * C:(e + 1) * C],
                            start=(k == 0), stop=(k == KD - 1),
                        )
                    for k in range(KD):
                        nc.tensor.matmul(
                            pu[:, :C], wgu[k][:, (j + 4) * P:(j + 5) * P],
                            tok[k][:, e * C:(e + 1) * C],
                            start=(k == 0), stop=(k == KD - 1),
                        )
                    sg = acts.tile([P, TT], dt.bfloat16, tag="sg", name=f"sg{e}_{j}")
                    nc.scalar.activation(sg[:, :C], pg[:, :C], f.Silu)
                    h = acts.tile([P, TT], dt.bfloat16, tag=f"h{j}", name=f"h{e}_{j}")
                    nc.vector.tensor_tensor(h[:, :C], sg[:, :C], pu[:, :C], mybir.AluOpType.mult)
                    hs.append(h)
                for m in range(KD):  # output feature chunks of HIDDEN
                    pd = psum.tile([P, TT], dt.float32, tag="pd", name=f"pd{e}_{m}")
                    for j in range(4):
                        nc.tensor.matmul(
                            pd[:, :C], wdl[j][:, m * P:(m + 1) * P], hs[j][:, :C],
                            start=(j == 0), stop=(j == 3),
                        )
                    yo = outs.tile([P, TT], dt.float32, tag="yo", name=f"yo{e}_{m}")
                    nc.any.tensor_copy(yo[:, :C], pd[:, :C])
                    nc.sync.dma_start(ymoe_d[m * P:(m + 1) * P, e * C:(e + 1) * C], yo[:, :C])

            # ---- shared expert shard ----
            for t_i in range(NT):
                tsl = slice(t_i * TT, (t_i + 1) * TT)
                hsh = []
                for j, (off, sz) in enumerate(FF_CH):
                    pg = psum.tile([P, TT], dt.float32, tag="pg", name=f"spg{t_i}_{j}")
                    pu = psum.tile([P, TT], dt.float32, tag="pu", name=f"spu{t_i}_{j}")
                    for k in range(KD):
                        nc.tensor.matmul(
                            pg[:sz, :], wg[k][:, off:off + sz], xt[k][:, tsl],
                            start=(k == 0), stop=(k == KD - 1),
                        )
                    for k in range(KD):
                        nc.tensor.matmul(
                            pu[:sz, :], wu[k][:, off:off + sz], xt[k][:, tsl],
                            start=(k == 0), stop=(k == KD - 1),
                        )
                    sg = acts.tile([P, TT], dt.bfloat16, tag="sg", name=f"ssg{t_i}_{j}")
                    nc.scalar.activation(sg[:sz, :], pg[:sz, :], f.Silu)
                    h = acts.tile([P, TT], dt.bfloat16, tag=f"h{j}", name=f"sh{t_i}_{j}")
                    nc.vector.tensor_tensor(h[:sz, :], sg[:sz, :], pu[:sz, :], mybir.AluOpType.mult)
                    hsh.append(h)
                for m in range(KD):
                    pd = psum.tile([P, TT], dt.float32, tag="pd", name=f"spd{t_i}_{m}")
                    for j, (off, sz) in enumerate(FF_CH):
                        nc.tensor.matmul(
                            pd[:], wdt[j][:sz, m * P:(m + 1) * P], hsh[j][:sz, :],
                            start=(j == 0), stop=(j == 2),
                        )
                    yo = outs.tile([P, TT], dt.float32, tag="yo", name=f"syo{t_i}_{m}")
                    nc.any.tensor_copy(yo[:], pd[:])
                    nc.sync.dma_start(ysh_d[m * P:(m + 1) * P, tsl], yo[:])
    _split_excess_waits(nc)
    return nc


def _route(x: np.ndarray, gate_w: np.ndarray):
    logits = x @ gate_w.T
    logits = logits.astype(np.float32)
    m = logits.max(axis=-1, keepdims=True)
    p = np.exp(logits - m)
    p /= p.sum(axis=-1, keepdims=True)
    sel = np.argsort(-p, axis=-1, kind="stable")[:, :TOP_K]
    rw = np.take_along_axis(p, sel, axis=-1)
    rw = rw / rw.sum(axis=-1, keepdims=True)
    idxs, wts = [], []
    for e in range(NUM_EXPERTS):
        mask = (sel == e).any(axis=-1)
        idx = np.nonzero(mask)[0]
        w = rw[idx][sel[idx] == e]
        idxs.append(idx)
        wts.append(w.astype(np.float32))
    return idxs, wts


def kernel(layer_input, gate_w, w_gate_up, w_down,
           shared_w_gate, shared_w_up, shared_w_down, shared_gate_w):
    B, S, D = layer_input.shape
    x = np.ascontiguousarray(np.asarray(layer_input, dtype=np.float32).reshape(-1, D))

    idxs, wts = _route(x, np.asarray(gate_w, dtype=np.float32))
    cmax = max(len(i) for i in idxs)
    C = max(P, ((cmax + P - 1) // P) * P)

    key = C
    if key not in _prog_cache:
        _prog_cache[key] = _build_program(C)
    nc = _prog_cache[key]

    xt = np.ascontiguousarray(x.T).astype(BF16)
    wgu_all = np.asarray(w_gate_up, dtype=np.float32).astype(BF16)
    wd_all = np.asarray(w_down, dtype=np.float32).astype(BF16)
    wg_t_all = np.ascontiguousarray(np.asarray(shared_w_gate, np.float32).T).astype(BF16)
    wu_t_all = np.ascontiguousarray(np.asarray(shared_w_up, np.float32).T).astype(BF16)
    wdt_all = np.ascontiguousarray(np.asarray(shared_w_down, np.float32).T).astype(BF16)

    in_maps = []
    for c in range(N_CORES):
        xe = np.zeros((2 * C, HIDDEN), dtype=BF16)
        for s_i, e in enumerate((2 * c, 2 * c + 1)):
            cnt = len(idxs[e])
            xe[s_i * C:s_i * C + cnt] = x[idxs[e]].astype(BF16)
        fsl = slice(c * FF_SH, (c + 1) * FF_SH)
        in_maps.append({
            "xt": xt,
            "xe": np.ascontiguousarray(xe.T),
            "wgu": np.ascontiguousarray(wgu_all[2 * c:2 * c + 2]),
            "wd": np.ascontiguousarray(wd_all[2 * c:2 * c + 2]),
            "wg_t": np.ascontiguousarray(wg_t_all[:, fsl]),
            "wu_t": np.ascontiguousarray(wu_t_all[:, fsl]),
            "wdt": np.ascontiguousarray(wdt_all[fsl, :]),
        })

    trace = bool(int(os.environ.get("BASS_MOE_TRACE", "0")))
    res = run_bass_kernel_spmd(
        nc, in_maps, core_ids=list(range(N_CORES)),
        trace=trace, trace_cores=list(range(N_CORES)) if trace else None,
    )
    kernel.last_results = res

    shared = np.zeros((HIDDEN, NTOK), dtype=np.float32)
    for c in range(N_CORES):
        shared += np.asarray(res.results[c]["y_sh"], dtype=np.float32)
    sig = 1.0 / (1.0 + np.exp(-(x @ np.asarray(shared_gate_w, np.float32).T)))
    out = shared.T * sig
    for e in range(NUM_EXPERTS):
        c, s_i = e // 2, e % 2
        cnt = len(idxs[e])
        if cnt == 0:
            continue
        ye = np.asarray(res.results[c]["y_moe"], dtype=np.float32)[:, s_i * C:s_i * C + cnt]
        out[idxs[e]] += wts[e][:, None] * ye.T
    return out.reshape(B, S, D).astype(np.float32)


# revision 4
# speedup vs baseline: 1.1647x; 1.1647x over previous
"""MoE (16 experts, top-2) + shared SwiGLU expert — Trainium2 Bass kernel.

Strategy (8 NeuronCores, SPMD):
  - Router runs on host (tiny: 2048x1024x16). Tokens are grouped by expert.
  - Expert-parallel: core c owns experts {2c, 2c+1}; host gathers the tokens
    routed to each expert (padded to capacity C) and ships them transposed
    (features-on-partitions) so no on-device transposes are needed.
  - Shared expert is FF-sharded: core c computes a 352-wide slice of the
    2816-wide SwiGLU FF; host sums the 8 partial down-projections.
  - All matmul inputs are cast to bf16 on host (fp32 accumulation in PSUM).
  - Host combine: out = sum(partials).T * sigmoid(x@sgw.T) + scatter(expert).
"""

import os

import numpy as np
import ml_dtypes

import concourse.bass as bass
import concourse.mybir as mybir
import concourse.tile as tile
from concourse.bass_utils import run_bass_kernel_spmd

HIDDEN = 1024
MOE_FF = 512
SHARED_FF = 2816
NUM_EXPERTS = 16
TOP_K = 2
N_CORES = 8
NTOK = 2048
FF_SH = SHARED_FF // N_CORES  # 352
P = 128
KD = HIDDEN // P  # 8 contraction chunks over hidden
FF_CH = [(0, 128), (128, 128), (256, 96)]  # shared-FF shard chunking (352)
TT = 512  # token tile (PSUM free-dim limit)
NT = NTOK // TT

BF16 = ml_dtypes.bfloat16

_prog_cache: dict = {}


def _split_excess_waits(nc: bass.Bass) -> None:
    """This container's walrus accepts at most 1 sync-wait per instruction
    (2 on EventSemaphore), but Tile's tail barrier can emit more; split the
    excess onto preceding EventSemaphore instructions on the same engine."""
    for fn in nc.m.functions:
        for blk in fn.blocks:
            out = []
            for ins in blk.instructions:
                si = ins.sync_info
                cap = 2 if isinstance(ins, mybir.InstEventSemaphore) else 1
                if si is not None and len(si.on_wait) > cap:
                    waits = list(si.on_wait)
                    excess, keep = waits[:-cap], waits[-cap:]
                    for i in range(0, len(excess), 2):
                        ev = mybir.InstEventSemaphore(
                            name=nc.get_next_instruction_name(), ins=[], outs=[])
                        ev.engine = ins.engine
                        ev.sync_info = mybir.SyncInfo(
                            on_wait=excess[i:i + 2], on_update=[])
                        nc.register_instruction(ev)
                        out.append(ev)
                    si.on_wait = keep
                out.append(ins)
            blk.instructions[:] = out


def _build_program(C: int) -> bass.Bass:
    """Per-core program. C = per-expert token capacity (multiple of 32)."""
    nc = bass.Bass()
    dt = mybir.dt
    f = mybir.ActivationFunctionType

    xt_d = nc.dram_tensor("xt", [HIDDEN, NTOK], dt.bfloat16, kind="ExternalInput")
    xe_d = nc.dram_tensor("xe", [HIDDEN, 2 * C], dt.bfloat16, kind="ExternalInput")
    wgu_d = nc.dram_tensor("wgu", [2, HIDDEN, 2 * MOE_FF], dt.bfloat16, kind="ExternalInput")
    wd_d = nc.dram_tensor("wd", [2, MOE_FF, HIDDEN], dt.bfloat16, kind="ExternalInput")
    wg_d = nc.dram_tensor("wg_t", [HIDDEN, FF_SH], dt.bfloat16, kind="ExternalInput")
    wu_d = nc.dram_tensor("wu_t", [HIDDEN, FF_SH], dt.bfloat16, kind="ExternalInput")
    wdt_d = nc.dram_tensor("wdt", [FF_SH, HIDDEN], dt.bfloat16, kind="ExternalInput")
    ymoe_d = nc.dram_tensor("y_moe", [HIDDEN, 2 * C], dt.bfloat16, kind="ExternalOutput")
    ysh_d = nc.dram_tensor("y_sh", [HIDDEN, NTOK], dt.bfloat16, kind="ExternalOutput")

    with tile.TileContext(nc) as tc:
        with (
            tc.tile_pool(name="res", bufs=1) as res,
            tc.tile_pool(name="wmoe", bufs=2) as wmoe,
            tc.tile_pool(name="acts", bufs=3) as acts,
            tc.tile_pool(name="outs", bufs=2) as outs,
            tc.tile_pool(name="psum", bufs=2, space="PSUM") as psum,
        ):
            # ---- loads, in consumption order; one batched DMA per tensor ----
            xe = res.tile([P, KD, 2 * C], dt.bfloat16, tag="xe", name="xe")
            nc.sync.dma_start(xe[:], xe_d.rearrange("(k p) t -> p k t", p=P))
            wgu = []
            wdl = []
            for e in range(2):
                wt = wmoe.tile([P, KD, 2 * MOE_FF], dt.bfloat16, tag=f"wgu{e}", name=f"wgu{e}")
                src = wgu_d[e].rearrange("(k p) f -> p k f", p=P)
                nc.sync.dma_start(wt[:, 0:4, :], src[:, 0:4, :])
                nc.sync.dma_start(wt[:, 4:8, :], src[:, 4:8, :])
                wgu.append(wt)
                dl = wmoe.tile([P, MOE_FF // P, HIDDEN], dt.bfloat16, tag=f"wd{e}", name=f"wd{e}")
                nc.sync.dma_start(dl[:], wd_d[e].rearrange("(j p) d -> p j d", p=P))
                wdl.append(dl)
            xt = res.tile([P, KD, NTOK], dt.bfloat16, tag="xt", name="xt")
            nc.sync.dma_start(xt[:], xt_d.rearrange("(k p) t -> p k t", p=P))
            wg = res.tile([P, KD, FF_SH], dt.bfloat16, tag="wg", name="wg")
            nc.sync.dma_start(wg[:], wg_d.rearrange("(k p) t -> p k t", p=P))
            wu = res.tile([P, KD, FF_SH], dt.bfloat16, tag="wu", name="wu")
            nc.sync.dma_start(wu[:], wu_d.rearrange("(k p) t -> p k t", p=P))
            wdt = []
            for j, (off, sz) in enumerate(FF_CH):
                t = res.tile([P, HIDDEN], dt.bfloat16, tag=f"wdt{j}", name=f"wdt{j}")
                nc.sync.dma_start(t[:sz, :], wdt_d[off:off + sz, :])
                wdt.append(t)

            ymoe_r = ymoe_d.rearrange("(m p) t -> p m t", p=P)
            ysh_r = ysh_d.rearrange("(m p) t -> p m t", p=P)

            # ---- MoE experts (2 per core) ----
            for e in range(2):
                tok = xe[:, :, e * C:(e + 1) * C]
                hs = []
                for j in range(4):  # gate/up feature pairs (j, j+4) of 2*MOE_FF
                    pg = psum.tile([P, TT], dt.float32, tag="pg", name=f"pg{e}_{j}")
                    pu = psum.tile([P, TT], dt.float32, tag="pu", name=f"pu{e}_{j}")
                    for k in range(KD):
                        nc.tensor.matmul(
                            pg[:, :C], wgu[e][:, k, j * P:(j + 1) * P], tok[:, k, :],
                            start=(k == 0), stop=(k == KD - 1),
                        )
                    for k in range(KD):
                        nc.tensor.matmul(
                            pu[:, :C], wgu[e][:, k, (j + 4) * P:(j + 5) * P], tok[:, k, :],
                            start=(k == 0), stop=(k == KD - 1),
                        )
                    sg = acts.tile([P, TT], dt.bfloat16, tag="sg", name=f"sg{e}_{j}")
                    nc.scalar.activation(sg[:, :C], pg[:, :C], f.Silu)
                    h = acts.tile([P, TT], dt.bfloat16, tag=f"h{j}", name=f"h{e}_{j}")
                    nc.vector.tensor_tensor(h[:, :C], sg[:, :C], pu[:, :C], mybir.AluOpType.mult)
                    hs.append(h)
                yo = outs.tile([P, KD, C], dt.bfloat16, tag="ymoe", name=f"ymoe{e}")
                for m in range(KD):  # output feature chunks of HIDDEN
                    pd = psum.tile([P, TT], dt.float32, tag="pd", name=f"pd{e}_{m}")
                    for j in range(4):
                        nc.tensor.matmul(
                            pd[:, :C], wdl[e][:, j, m * P:(m + 1) * P], hs[j][:, :C],
                            start=(j == 0), stop=(j == 3),
                        )
                    nc.scalar.activation(yo[:, m, :], pd[:, :C], f.Copy)
                nc.gpsimd.dma_start(ymoe_r[:, :, e * C:(e + 1) * C], yo[:])

            # ---- shared expert shard ----
            for t_i in range(NT):
                tsl = slice(t_i * TT, (t_i + 1) * TT)
                hsh = []
                for j, (off, sz) in enumerate(FF_CH):
                    pg = psum.tile([P, TT], dt.float32, tag="pg", name=f"spg{t_i}_{j}")
                    pu = psum.tile([P, TT], dt.float32, tag="pu", name=f"spu{t_i}_{j}")
                    for k in range(KD):
                        nc.tensor.matmul(
                            pg[:sz, :], wg[:, k, off:off + sz], xt[:, k, tsl],
                            start=(k == 0), stop=(k == KD - 1),
                        )
                    for k in range(KD):
                        nc.tensor.matmul(
                            pu[:sz, :], wu[:, k, off:off + sz], xt[:, k, tsl],
                            start=(k == 0), stop=(k == KD - 1),
                        )
                    sg = acts.tile([P, TT], dt.bfloat16, tag="sg", name=f"ssg{t_i}_{j}")
                    nc.scalar.activation(sg[:sz, :], pg[:sz, :], f.Silu)
                    h = acts.tile([P, TT], dt.bfloat16, tag=f"h{j}", name=f"sh{t_i}_{j}")
                    nc.vector.tensor_tensor(h[:sz, :], sg[:sz, :], pu[:sz, :], mybir.AluOpType.mult)
                    hsh.append(h)
                ys = outs.tile([P, KD, TT], dt.bfloat16, tag="ysh", name=f"ysh{t_i}")
                for m in range(KD):
                    pd = psum.tile([P, TT], dt.float32, tag="pd", name=f"spd{t_i}_{m}")
                    for j, (off, sz) in enumerate(FF_CH):
                        nc.tensor.matmul(
                            pd[:], wdt[j][:sz, m * P:(m + 1) * P], hsh[j][:sz, :],
                            start=(j == 0), stop=(j == 2),
                        )
                    nc.vector.tensor_copy(ys[:, m, :], pd[:])
                nc.gpsimd.dma_start(ysh_r[:, :, tsl], ys[:])
    _split_excess_waits(nc)
    return nc


def _route(x: np.ndarray, gate_w: np.ndarray):
    logits = x @ gate_w.T
    logits = logits.astype(np.float32)
    m = logits.max(axis=-1, keepdims=True)
    p = np.exp(logits - m)
    p /= p.sum(axis=-1, keepdims=True)
    sel = np.argsort(-p, axis=-1, kind="stable")[:, :TOP_K]
    rw = np.take_along_axis(p, sel, axis=-1)
    rw = rw / rw.sum(axis=-1, keepdims=True)
    idxs, wts = [], []
    for e in range(NUM_EXPERTS):
        mask = (sel == e).any(axis=-1)
        idx = np.nonzero(mask)[0]
        w = rw[idx][sel[idx] == e]
        idxs.append(idx)
        wts.append(w.astype(np.float32))
    return idxs, wts


def kernel(layer_input, gate_w, w_gate_up, w_down,
           shared_w_gate, shared_w_up, shared_w_down, shared_gate_w):
    B, S, D = layer_input.shape
    x = np.ascontiguousarray(np.asarray(layer_input, dtype=np.float32).reshape(-1, D))

    idxs, wts = _route(x, np.asarray(gate_w, dtype=np.float32))
    cmax = max(len(i) for i in idxs)
    C = max(32, ((cmax + 31) // 32) * 32)

    key = C
    if key not in _prog_cache:
        _prog_cache[key] = _build_program(C)
    nc = _prog_cache[key]

    xt = np.ascontiguousarray(x.T).astype(BF16)
    wgu_all = np.asarray(w_gate_up, dtype=np.float32).astype(BF16)
    wd_all = np.asarray(w_down, dtype=np.float32).astype(BF16)
    wg_t_all = np.ascontiguousarray(np.asarray(shared_w_gate, np.float32).T).astype(BF16)
    wu_t_all = np.ascontiguousarray(np.asarray(shared_w_up, np.float32).T).astype(BF16)
    wdt_all = np.ascontiguousarray(np.asarray(shared_w_down, np.float32).T).astype(BF16)

    in_maps = []
    for c in range(N_CORES):
        xe = np.zeros((2 * C, HIDDEN), dtype=BF16)
        for s_i, e in enumerate((2 * c, 2 * c + 1)):
            cnt = len(idxs[e])
            xe[s_i * C:s_i * C + cnt] = x[idxs[e]].astype(BF16)
        fsl = slice(c * FF_SH, (c + 1) * FF_SH)
        in_maps.append({
            "xt": xt,
            "xe": np.ascontiguousarray(xe.T),
            "wgu": np.ascontiguousarray(wgu_all[2 * c:2 * c + 2]),
            "wd": np.ascontiguousarray(wd_all[2 * c:2 * c + 2]),
            "wg_t": np.ascontiguousarray(wg_t_all[:, fsl]),
            "wu_t": np.ascontiguousarray(wu_t_all[:, fsl]),
            "wdt": np.ascontiguousarray(wdt_all[fsl, :]),
        })

    trace = bool(int(os.environ.get("BASS_MOE_TRACE", "0")))
    res = run_bass_kernel_spmd(
        nc, in_maps, core_ids=list(range(N_CORES)),
        trace=trace, trace_cores=list(range(N_CORES)) if trace else None,
    )
    kernel.last_results = res

    shared = np.zeros((HIDDEN, NTOK), dtype=np.float32)
    for c in range(N_CORES):
        shared += np.asarray(res.results[c]["y_sh"]).astype(np.float32)
    sig = 1.0 / (1.0 + np.exp(-(x @ np.asarray(shared_gate_w, np.float32).T)))
    out = shared.T * sig
    for e in range(NUM_EXPERTS):
        c, s_i = e // 2, e % 2
        cnt = len(idxs[e])
        if cnt == 0:
            continue
        ye = np.asarray(res.results[c]["y_moe"]).astype(np.float32)[:, s_i * C:s_i * C + cnt]
        out[idxs[e]] += wts[e][:, None] * ye.T
    return out.reshape(B, S, D).astype(np.float32)
